# revision 5
# baseline (speedup 1.0000x reference)
"""Fused quantized Conv2D + BatchNorm via 1-D Winograd F(2,3) on Trainium2.

Reference computation (shapes hardcoded):
  x:      [32, 128, 56, 56] f32    activations in [0, 1)
  weight: [256, 128, 3, 3]  f32
  bias/gamma/beta/running_mean/running_var: [256] f32

  xq = round(clip(x,0,4) * 255/4) * (4/255)          (8-bit act quant)
  wq = DoReFa 8-bit weight quant -> values (2k-255)/255
  y  = conv2d(xq, wq, stride 1, pad 1)               NCHW/OIHW
  out = y * inv + shift

Kernel strategy:
  - Data-parallel over batch: core i handles images [4i, 4i+4).
  - Winograd F(2,3) applied over the H axis: each pair of output rows
    (one "htile") costs 4 position-matmuls x 3 horizontal taps = 12
    matmuls per (chunk, couttile) instead of the 18 a direct conv
    needs: 1.5x less PE work.  All values stay exact in fp16:
    d = round(63.75*x) in {0..255}; V = B^T d ints <= 510; U = G b
    half-ints <= 382.5 scaled by the folded BN scale.
  - Inverse transform y_e = m0+m1+m2+sh, y_o = m1-m2-m3+sh is spread
    over three engines (PSUM has one DVE read port, so each op reads
    at most one PSUM operand):
      ACT:   m1p = m1 + sh -> fp16,  m2p = m2 -> fp16
      DVE:   s01 = m0(PSUM) + m1p,   u = m1p - m2p (2x fp16),
             y_o = u - m3(PSUM) -> f32
      GpSimd: y_e = s01 + m2p -> f32
  - Quantization runs on ACT (Copy with scale=63.75, bias=2^23 forcing
    RNE to integers, then Copy with bias=-2^23 casting to fp16 d).
  - Forward transform: 4 fp16 tensor_tensor ops per chunk on DVE at
    2x packing (V_j tiles carry the 58-wide column halo so the 3 kw
    taps are strided views).
"""

import sys
import types

import numpy as np
import ml_dtypes

import concourse.bacc as bacc
import concourse.tile as tile
from concourse import mybir
from concourse.bass_utils import run_bass_kernel_spmd
from concourse.vector_clock import ScopedClock


def _ensure_axon_hooks_shim():
    import antenv
    if hasattr(antenv, "axon_hooks"):
        return
    mod = types.ModuleType("antenv.axon_hooks")
    _hook = [None]
    mod.set_axon_ntff_profile_hook = lambda h: _hook.__setitem__(0, h)
    mod.get_axon_ntff_profile_hook = lambda: _hook[0]
    sys.modules["antenv.axon_hooks"] = mod
    antenv.axon_hooks = mod


_ensure_axon_hooks_shim()


class _FastBacc(bacc.Bacc):
    """Bacc whose constructor-time all-engine barrier is skipped."""

    _skip_one_barrier = True

    def all_engine_barrier(self, *, sem_only: bool = False):
        if self._skip_one_barrier:
            self._skip_one_barrier = False
            return
        super().all_engine_barrier(sem_only=sem_only)


class _FastExitTileContext(tile.TileContext):
    """TileContext with a cheaper exit ceremony (see baseline notes)."""

    def _drain_and_barrier(self, tick_clock, wait_clock):
        drain_inst = self.nc.gpsimd.drain()
        wait_clock.add_sem_waits(
            drain_inst.ins, ScopedClock({None: tick_clock.global_clock})
        )
        popped = self.nc._tile_sem_poison_stack.pop()
        assert popped is self._sem_poison
        self.nc.clear_and_free_semaphores(list(self.sems.allocated().values()))


N_CORES = 8
N_BATCH = 32
IMGS = N_BATCH // N_CORES
CIN = 128
COUT = 256
H = W = 56
HW = H * W
WP = 58          # padded column width (col halo for the 3 kw taps)
HP = 58          # padded row count (rows -1 .. 56)
HT = 28          # htiles per image (2 output rows each)
CH = 7           # htiles per matmul chunk
NCH = HT // CH   # 4 chunks
NMM = CH * W     # 392 free elements per matmul
COUT_TILES = 2
NPOS = 4         # Winograd positions j
KW = 3
N_WARM_MM = 22

# disjoint input slabs: slab c covers x rows [SLAB_LO[c], SLAB_HI[c])
SLAB_LO = [0, 15, 29, 43]
SLAB_HI = [15, 29, 43, 56]

MAGIC = np.float32(2.0**23)

TRACE = False
TRACE_DIR = None
LAST_RESULT = None
SIM_SAFE_EXIT = False  # use the stock (slow, race-clean) exit for CoreSim

_cached_nc = None


def _build():
    f32 = mybir.dt.float32
    f16 = mybir.dt.float16
    mult = mybir.AluOpType.mult
    add = mybir.AluOpType.add
    sub = mybir.AluOpType.subtract
    Copy = mybir.ActivationFunctionType.Copy
    Ident = mybir.ActivationFunctionType.Identity

    ctx_cls = tile.TileContext if SIM_SAFE_EXIT else _FastExitTileContext
    bacc_cls = bacc.Bacc if SIM_SAFE_EXIT else _FastBacc
    nc = bacc_cls("TRN2", target_bir_lowering=False, debug=False,
                  num_devices=N_CORES)
    xs = nc.dram_tensor("xs", [IMGS, CIN, HW], f32, kind="ExternalInput").ap()
    # Winograd weights, column idx = ((j*3+kw)*2 + ct)*128 + cout_local
    wb = nc.dram_tensor("wb", [CIN, NPOS * KW * COUT], f16,
                        kind="ExternalInput").ap()
    shd = nc.dram_tensor("shd", [128, COUT_TILES], f32, kind="ExternalInput").ap()
    ys = nc.dram_tensor("ys", [IMGS, COUT, HW], f32, kind="ExternalOutput").ap()

    with ctx_cls(nc) as tc:
        with (
            tc.tile_pool(name="wpool", bufs=1) as wpool,
            tc.tile_pool(name="ppool", bufs=1) as ppool,
            tc.tile_pool(name="stpool", bufs=4) as stpool,
            tc.tile_pool(name="dpool", bufs=1) as dpool,
            tc.tile_pool(name="vpool", bufs=1) as vpool,
            tc.tile_pool(name="mpool", bufs=3) as mpool,
            tc.tile_pool(name="opool", bufs=5) as opool,
            tc.tile_pool(name="pspool", bufs=1, space="PSUM") as pspool,
        ):
            # ACT warm-up: the activation-table load overlaps the first DMA.
            warm = ppool.tile([128, 1], f32, tag="warm")
            nc.vector.memset(warm[:], 0.0)
            nc.scalar.activation(warm[:], warm[:], Copy, bias=0.0)

            # HAM warm-up dummies.
            dummy = wpool.tile([128, 128 + NMM], f16, tag="dummy")
            nc.vector.memset(dummy[:], 0.0)
            dps = pspool.tile([128, NMM], f32, tag="ps0", name="ps_dummy")
            for i in range(N_WARM_MM):
                nc.tensor.matmul(dps[:], dummy[:, :128], dummy[:, 128:],
                                 start=True, stop=True)

            # shift vector [128, 2]
            shb = ppool.tile([128, COUT_TILES], f32, tag="shb")
            nc.gpsimd.dma_start(shb[:], shd[:])

            # d tiles (2 image slots), fp16; only the pad borders (row 0,
            # row 57, col 0, col 57) need zeroing -- interiors are fully
            # rewritten every image.
            dts = []
            for s in range(2):
                t = dpool.tile([CIN, HP * WP], f16, tag=f"d{s}", name=f"d{s}")
                eng = nc.gpsimd if s == 0 else nc.vector
                t3 = t.rearrange("p (h w) -> p h w", w=WP)
                eng.memset(t3[:, 0, :], 0.0)       # top pad row
                eng.memset(t3[:, HP - 1, :], 0.0)  # bottom pad row
                eng.memset(t3[:, :, 0:1], 0.0)     # left pad col
                eng.memset(t3[:, :, WP - 1:WP], 0.0)  # right pad col
                dts.append(t)
            # V tiles (4 positions x 2 slots)
            vts = [[vpool.tile([CIN, HT * WP], f16, tag=f"v{j}_{s}",
                               name=f"v{j}_{s}")
                    for j in range(NPOS)] for s in range(2)]

            # weights: j=0 block first (needed by the first matmul group)
            w1 = wpool.tile([CIN, 6 * 128], f16, tag="w1")
            w2 = wpool.tile([CIN, 18 * 128], f16, tag="w2")

            def wmat(j, kw, ct):
                idx = (j * KW + kw) * 2 + ct
                if j == 0:
                    return w1[:, idx * 128:(idx + 1) * 128]
                return w2[:, (idx - 6) * 128:(idx - 6 + 1) * 128]

            dma_engs = [nc.sync, nc.scalar, nc.gpsimd]

            # ---- helpers ------------------------------------------------
            def dma_slab(n, c, split):
                lo, hi = SLAB_LO[c], SLAB_HI[c]
                nr = hi - lo
                st = stpool.tile([CIN, nr * W], f32, tag="st",
                                 name=f"st{n}_{c}")
                if split:
                    h = (nr // 2) * W
                    nc.sync.dma_start(st[:, :h], xs[n][:, lo * W:lo * W + h])
                    nc.scalar.dma_start(st[:, h:],
                                        xs[n][:, lo * W + h:hi * W])
                else:
                    (nc.sync if c % 2 == 0 else nc.scalar).dma_start(
                        st[:], xs[n][:, lo * W:hi * W])
                return st

            def quant_scale(n, c, st):
                # gpsimd: a = 63.75*x + 2^23 in-place (f32 RNE to integer)
                nc.gpsimd.tensor_scalar(st[:], st[:], 63.75, float(MAGIC),
                                        op0=mult, op1=add)

            def quant_conv(n, c, st):
                # d = a - 2^23 cast to fp16, into padded d rows.  Even
                # slabs on ACT, odd slabs on gpsimd to balance load.
                s = n % 2
                lo, hi = SLAB_LO[c], SLAB_HI[c]
                nr = hi - lo
                d3 = dts[s].rearrange("p (h w) -> p h w", w=WP)
                dst = d3[:, 1 + lo: 1 + hi, 1:1 + W]
                src = st.rearrange("p (h w) -> p h w", w=W)[:, :nr, :]
                if c % 2 == 0:
                    nc.scalar.activation(dst, src, Copy, bias=float(-MAGIC))
                else:
                    nc.gpsimd.tensor_scalar_add(dst, src, float(-MAGIC))

            def fwd(n, c, nt=CH, js=(0, 1, 2, 3)):
                # V_j[i] for htiles i in [i0, i0+nt) from padded rows
                # 2*i0..2*(i0+nt)+1.  d4[p, i, r, w]: padded row 2i+r.
                s = n % 2
                d4 = dts[s].rearrange("p (i r w) -> p i r w", r=2, w=WP)
                v = [vts[s][j].rearrange("p (i w) -> p i w", w=WP)
                     for j in range(NPOS)]
                i0 = CH * c
                e0 = d4[:, i0:i0 + nt, 0, :]        # rows 2i
                e1 = d4[:, i0 + 1:i0 + nt + 1, 0, :]  # rows 2i+2
                o0 = d4[:, i0:i0 + nt, 1, :]        # rows 2i+1
                o1 = d4[:, i0 + 1:i0 + nt + 1, 1, :]  # rows 2i+3
                srcs = {0: (e0, e1, sub), 1: (o0, e1, add),
                        2: (e1, o0, sub), 3: (o0, o1, sub)}
                for j in js:
                    a, b, op = srcs[j]
                    nc.vector.tensor_tensor(v[j][:, i0:i0 + nt, :], a, b, op)

            ot_tiles = {}
            ps_seq = 0

            def compute(n, c, ct):
                nonlocal ps_seq
                s = n % 2
                half = c // 2
                key = (ct, half)
                if c % 2 == 0:
                    ot_tiles[key] = opool.tile([128, 2 * CH * 2 * W], f32,
                                               tag=f"ot{ct}",
                                               name=f"ot{n}_{ct}_{half}")
                ot = ot_tiles[key]
                i0 = CH * c
                ps = []
                bank0 = (ps_seq % 2) * 4
                ps_seq += 1
                for j in range(NPOS):
                    p = pspool.tile([128, NMM], f32, tag=f"ps{bank0 + j}",
                                    name=f"ps{n}_{c}_{ct}_{j}")
                    ps.append(p)
                    vv = vts[s][j].rearrange("p (i w) -> p i w", w=WP)
                    for kw in range(KW):
                        nc.tensor.matmul(
                            p[:], wmat(j, kw, ct),
                            vv[:, i0:i0 + CH, kw:kw + W],
                            start=(kw == 0), stop=(kw == KW - 1),
                        )
                m0p = mpool.tile([128, NMM], f16, tag="m0p",
                                 name=f"m0p{n}_{c}_{ct}")
                nc.scalar.activation(m0p[:], ps[0][:], Copy)
                m1p = mpool.tile([128, NMM], f16, tag="m1p",
                                 name=f"m1p{n}_{c}_{ct}")
                nc.scalar.activation(m1p[:], ps[1][:], Ident,
                                     bias=shb[:, ct:ct + 1])
                m2p = mpool.tile([128, NMM], f16, tag="m2p",
                                 name=f"m2p{n}_{c}_{ct}")
                nc.scalar.activation(m2p[:], ps[2][:], Copy)
                s01 = mpool.tile([128, NMM], f16, tag="s01",
                                 name=f"s01{n}_{c}_{ct}")
                nc.vector.tensor_tensor(s01[:], m0p[:], m1p[:], add)
                u = mpool.tile([128, NMM], f16, tag="u",
                               name=f"u{n}_{c}_{ct}")
                nc.vector.tensor_tensor(u[:], m1p[:], m2p[:], sub)
                # ot view [p, chunkhalf, htile, evenodd, w]
                o4 = ot.rearrange("p (k i r w) -> p k i r w", k=2, r=2, w=W)
                nc.gpsimd.tensor_tensor(o4[:, c % 2, :, 0, :], s01[:],
                                        m2p[:], add)
                nc.vector.tensor_tensor(o4[:, c % 2, :, 1, :], u[:],
                                        ps[3][:], sub)

            store_seq = 0
            CHW = CH * 2 * W  # 784, one chunk's output elements

            def store(n, c, ct, whole_half):
                nonlocal store_seq
                half = c // 2
                ot = ot_tiles[(ct, half)]
                if whole_half:
                    src = ot[:]
                    dst = ys[n, ct * 128:(ct + 1) * 128,
                             half * 2 * CHW:(half + 1) * 2 * CHW]
                    dma_engs[store_seq % 3].dma_start(dst, src)
                else:
                    # final image: split each chunk store across two queues
                    src = ot[:, (c % 2) * CHW:(c % 2 + 1) * CHW]
                    h = CHW // 2
                    o = c * CHW
                    cs = ct * 128
                    dma_engs[store_seq % 3].dma_start(
                        ys[n, cs:cs + 128, o:o + h], src[:, :h])
                    dma_engs[(store_seq + 1) % 3].dma_start(
                        ys[n, cs:cs + 128, o + h:o + 2 * h], src[:, h:])
                store_seq += 1

            # ---- emission ----------------------------------------------
            # image 0 prep.  HWDGE rings are FIFO per engine, so the trigger
            # order IS the priority order: slab 0 halves first on both rings,
            # then the weights, then the remaining slabs.
            sts = {}
            sts[(0, 0)] = dma_slab(0, 0, split=True)
            nc.sync.dma_start(w1[:], wb[:, :6 * 128])
            nc.scalar.dma_start(w2[:], wb[:, 6 * 128:])
            sts[(0, 1)] = dma_slab(0, 1, split=False)   # scalar
            sts[(0, 2)] = dma_slab(0, 2, split=False)   # sync
            sts[(0, 3)] = dma_slab(0, 3, split=False)   # scalar
            for c in range(NCH):
                quant_scale(0, c, sts[(0, c)])
                quant_conv(0, c, sts[(0, c)])
                fwd(0, c)

            for n in range(IMGS):
                last = n == IMGS - 1
                for c in range(NCH):
                    if not last and c < 2:
                        # next image's input slabs, two per compute chunk
                        sts[(n + 1, 2 * c)] = dma_slab(n + 1, 2 * c, False)
                        sts[(n + 1, 2 * c + 1)] = dma_slab(n + 1, 2 * c + 1,
                                                           False)
                    compute(n, c, 0)
                    if not last:
                        quant_scale(n + 1, c, sts[(n + 1, c)])
                        if c == 2:
                            fwd(n + 1, 0, nt=2 * CH, js=(0, 1))
                    compute(n, c, 1)
                    if not last:
                        quant_conv(n + 1, c, sts[(n + 1, c)])
                        if c == 2:
                            fwd(n + 1, 0, nt=2 * CH, js=(2, 3))
                        elif c == 3:
                            fwd(n + 1, 2, nt=2 * CH)
                    if last:
                        store(n, c, 0, whole_half=False)
                        store(n, c, 1, whole_half=False)
                    elif c % 2 == 1:
                        store(n, c, 0, whole_half=True)
                        store(n, c, 1, whole_half=True)

    nc.compile()
    return nc


def _get_nc():
    global _cached_nc
    if _cached_nc is None:
        _cached_nc = _build()
    return _cached_nc


def _host_prep(weight, bias, gamma, beta, running_mean, running_var):
    # DoReFa weight quantization mirroring the reference.
    wt = np.tanh(weight)
    wt = wt / np.abs(wt).max()
    k = np.round(wt * np.float32(127.5) + np.float32(127.5)).astype(np.float32)
    b_int = np.float32(2.0) * k - np.float32(255.0)  # [COUT, CIN, 3, 3]

    inv = gamma * (np.float32(1.0) / np.sqrt(running_var + np.float32(1e-5)))
    shift = beta - running_mean * inv + bias
    scale = inv * np.float32(4.0 / 65025.0)

    g0 = b_int[:, :, 0, :]
    g1 = b_int[:, :, 1, :]
    g2 = b_int[:, :, 2, :]
    U = np.stack([g0, (g0 + g1 + g2) * np.float32(0.5),
                  (g0 - g1 + g2) * np.float32(0.5), g2], axis=0)
    U = U * scale[None, :, None, None]            # [4, COUT, CIN, KW]
    wb_host = np.zeros((CIN, NPOS * KW * COUT), dtype=np.float16)
    for j in range(NPOS):
        for kw in range(KW):
            for ct in range(COUT_TILES):
                idx = (j * KW + kw) * 2 + ct
                wb_host[:, idx * 128:(idx + 1) * 128] = (
                    U[j, ct * 128:(ct + 1) * 128, :, kw].T
                )
    sh_host = np.ascontiguousarray(shift.reshape(COUT_TILES, 128).T)
    return wb_host, sh_host


def kernel(x, weight, bias, gamma, beta, running_mean, running_var):
    global LAST_RESULT
    x = np.asarray(x, dtype=np.float32)
    wb_host, sh_host = _host_prep(
        np.asarray(weight, dtype=np.float32),
        np.asarray(bias, dtype=np.float32),
        np.asarray(gamma, dtype=np.float32),
        np.asarray(beta, dtype=np.float32),
        np.asarray(running_mean, dtype=np.float32),
        np.asarray(running_var, dtype=np.float32))

    nc = _get_nc()
    in_maps = []
    for core in range(N_CORES):
        xs_c = np.ascontiguousarray(
            x[core * IMGS:(core + 1) * IMGS].reshape(IMGS, CIN, HW)
        )
        in_maps.append({"xs": xs_c, "wb": wb_host, "shd": sh_host})

    res = run_bass_kernel_spmd(nc, in_maps, list(range(N_CORES)), trace=TRACE,
                               tmpdir=TRACE_DIR)
    LAST_RESULT = res

    out = np.empty((N_BATCH, COUT, H, W), dtype=np.float32)
    for core in range(N_CORES):
        out[core * IMGS:(core + 1) * IMGS] = (
            res.results[core]["ys"].reshape(IMGS, COUT, H, W)
        )
    return out


# revision 6
# speedup vs baseline: 1.8133x; 1.8133x over previous
"""Fused quantized Conv2D + BatchNorm via 1-D Winograd F(2,3) on Trainium2.

Reference computation (shapes hardcoded):
  x:      [32, 128, 56, 56] f32    activations in [0, 1)
  weight: [256, 128, 3, 3]  f32
  bias/gamma/beta/running_mean/running_var: [256] f32

  xq = round(clip(x,0,4) * 255/4) * (4/255)          (8-bit act quant)
  wq = DoReFa 8-bit weight quant -> values (2k-255)/255
  y  = conv2d(xq, wq, stride 1, pad 1)               NCHW/OIHW
  out = y * inv + shift

Kernel strategy:
  - Data-parallel over batch: core i handles images [4i, 4i+4).
  - Winograd F(2,3) applied over the H axis: each pair of output rows
    (one "htile") costs 4 position-matmuls x 3 horizontal taps = 12
    matmuls per (chunk, couttile) instead of the 18 a direct conv
    needs: 1.5x less PE work.  All values stay exact in fp16:
    d = round(63.75*x) in {0..255}; V = B^T d ints <= 510; U = G b
    half-ints <= 382.5 scaled by the folded BN scale.
  - Inverse transform y_e = m0+m1+m2+sh, y_o = m1-m2-m3+sh is spread
    over three engines (PSUM has one DVE read port, so each op reads
    at most one PSUM operand):
      ACT:   m1p = m1 + sh -> fp16,  m2p = m2 -> fp16
      DVE:   s01 = m0(PSUM) + m1p,   u = m1p - m2p (2x fp16),
             y_o = u - m3(PSUM) -> f32
      GpSimd: y_e = s01 + m2p -> f32
  - Quantization runs on ACT (Copy with scale=63.75, bias=2^23 forcing
    RNE to integers, then Copy with bias=-2^23 casting to fp16 d).
  - Forward transform: 4 fp16 tensor_tensor ops per chunk on DVE at
    2x packing (V_j tiles carry the 58-wide column halo so the 3 kw
    taps are strided views).
"""

import sys
import types

import numpy as np
import ml_dtypes

import concourse.bacc as bacc
import concourse.tile as tile
from concourse import mybir
from concourse.bass_utils import run_bass_kernel_spmd
from concourse.vector_clock import ScopedClock


def _ensure_axon_hooks_shim():
    import antenv
    if hasattr(antenv, "axon_hooks"):
        return
    mod = types.ModuleType("antenv.axon_hooks")
    _hook = [None]
    mod.set_axon_ntff_profile_hook = lambda h: _hook.__setitem__(0, h)
    mod.get_axon_ntff_profile_hook = lambda: _hook[0]
    sys.modules["antenv.axon_hooks"] = mod
    antenv.axon_hooks = mod


_ensure_axon_hooks_shim()


class _FastBacc(bacc.Bacc):
    """Bacc whose constructor-time all-engine barrier is skipped."""

    _skip_one_barrier = True

    def all_engine_barrier(self, *, sem_only: bool = False):
        if self._skip_one_barrier:
            self._skip_one_barrier = False
            return
        super().all_engine_barrier(sem_only=sem_only)


class _FastExitTileContext(tile.TileContext):
    """TileContext with a cheaper exit ceremony (see baseline notes)."""

    def _drain_and_barrier(self, tick_clock, wait_clock):
        drain_inst = self.nc.gpsimd.drain()
        wait_clock.add_sem_waits(
            drain_inst.ins, ScopedClock({None: tick_clock.global_clock})
        )
        popped = self.nc._tile_sem_poison_stack.pop()
        assert popped is self._sem_poison
        self.nc.clear_and_free_semaphores(list(self.sems.allocated().values()))


N_CORES = 8
N_BATCH = 32
IMGS = N_BATCH // N_CORES
CIN = 128
COUT = 256
H = W = 56
HW = H * W
WP = 58          # padded column width (col halo for the 3 kw taps)
HP = 58          # padded row count (rows -1 .. 56)
HT = 28          # htiles per image (2 output rows each)
CH = 7           # htiles per matmul chunk
NCH = HT // CH   # 4 chunks
NMM = CH * W     # 392 free elements per matmul
COUT_TILES = 2
NPOS = 4         # Winograd positions j
KW = 3
N_WARM_MM = 22

# disjoint input slabs: slab c covers x rows [SLAB_LO[c], SLAB_HI[c])
SLAB_LO = [0, 15, 29, 43]
SLAB_HI = [15, 29, 43, 56]

MAGIC = np.float32(2.0**23)

TRACE = False
TRACE_DIR = None
LAST_RESULT = None
SIM_SAFE_EXIT = False  # use the stock (slow, race-clean) exit for CoreSim

_cached_nc = None


def _build():
    f32 = mybir.dt.float32
    f16 = mybir.dt.float16
    mult = mybir.AluOpType.mult
    add = mybir.AluOpType.add
    sub = mybir.AluOpType.subtract
    Copy = mybir.ActivationFunctionType.Copy
    Ident = mybir.ActivationFunctionType.Identity

    ctx_cls = tile.TileContext if SIM_SAFE_EXIT else _FastExitTileContext
    bacc_cls = bacc.Bacc if SIM_SAFE_EXIT else _FastBacc
    nc = bacc_cls("TRN2", target_bir_lowering=False, debug=False,
                  num_devices=N_CORES)
    xs = nc.dram_tensor("xs", [IMGS, CIN, HW], f32, kind="ExternalInput").ap()
    # Winograd weights, column idx = ((j*3+kw)*2 + ct)*128 + cout_local
    wb = nc.dram_tensor("wb", [CIN, NPOS * KW * COUT], f16,
                        kind="ExternalInput").ap()
    shd = nc.dram_tensor("shd", [128, COUT_TILES], f32, kind="ExternalInput").ap()
    ys = nc.dram_tensor("ys", [IMGS, COUT, HW], f32, kind="ExternalOutput").ap()

    with ctx_cls(nc) as tc:
        with (
            tc.tile_pool(name="wpool", bufs=1) as wpool,
            tc.tile_pool(name="ppool", bufs=1) as ppool,
            tc.tile_pool(name="stpool", bufs=4) as stpool,
            tc.tile_pool(name="dpool", bufs=1) as dpool,
            tc.tile_pool(name="vpool", bufs=1) as vpool,
            tc.tile_pool(name="mpool", bufs=3) as mpool,
            tc.tile_pool(name="opool", bufs=5) as opool,
            tc.tile_pool(name="pspool", bufs=1, space="PSUM") as pspool,
        ):
            # ACT warm-up: the activation-table load overlaps the first DMA.
            warm = ppool.tile([128, 1], f32, tag="warm")
            nc.vector.memset(warm[:], 0.0)
            nc.scalar.activation(warm[:], warm[:], Copy, bias=0.0)

            # HAM warm-up dummies.
            dummy = wpool.tile([128, 128 + NMM], f16, tag="dummy")
            nc.vector.memset(dummy[:], 0.0)
            dps = pspool.tile([128, NMM], f32, tag="ps0", name="ps_dummy")
            for i in range(N_WARM_MM):
                nc.tensor.matmul(dps[:], dummy[:, :128], dummy[:, 128:],
                                 start=True, stop=True)

            # shift vector [128, 2]
            shb = ppool.tile([128, COUT_TILES], f32, tag="shb")
            nc.gpsimd.dma_start(shb[:], shd[:])

            # d tiles (2 image slots), fp16; only the pad borders (row 0,
            # row 57, col 0, col 57) need zeroing -- interiors are fully
            # rewritten every image.
            dts = []
            for s in range(2):
                t = dpool.tile([CIN, HP * WP], f16, tag=f"d{s}", name=f"d{s}")
                eng = nc.gpsimd if s == 0 else nc.vector
                t3 = t.rearrange("p (h w) -> p h w", w=WP)
                eng.memset(t3[:, 0, :], 0.0)       # top pad row
                eng.memset(t3[:, HP - 1, :], 0.0)  # bottom pad row
                eng.memset(t3[:, :, 0:1], 0.0)     # left pad col
                eng.memset(t3[:, :, WP - 1:WP], 0.0)  # right pad col
                dts.append(t)
            # V tiles (4 positions x 2 slots)
            vts = [[vpool.tile([CIN, HT * WP], f16, tag=f"v{j}_{s}",
                               name=f"v{j}_{s}")
                    for j in range(NPOS)] for s in range(2)]

            # weights: j=0 block first (needed by the first matmul group)
            w1 = wpool.tile([CIN, 6 * 128], f16, tag="w1")
            w2 = wpool.tile([CIN, 18 * 128], f16, tag="w2")

            def wmat(j, kw, ct):
                idx = (j * KW + kw) * 2 + ct
                if j == 0:
                    return w1[:, idx * 128:(idx + 1) * 128]
                return w2[:, (idx - 6) * 128:(idx - 6 + 1) * 128]

            dma_engs = [nc.sync, nc.scalar, nc.gpsimd]

            # ---- helpers ------------------------------------------------
            def dma_slab(n, c, split):
                lo, hi = SLAB_LO[c], SLAB_HI[c]
                nr = hi - lo
                st = stpool.tile([CIN, nr * W], f32, tag="st",
                                 name=f"st{n}_{c}")
                if split:
                    h = (nr // 2) * W
                    nc.sync.dma_start(st[:, :h], xs[n][:, lo * W:lo * W + h])
                    nc.scalar.dma_start(st[:, h:],
                                        xs[n][:, lo * W + h:hi * W])
                else:
                    (nc.sync if c % 2 == 0 else nc.scalar).dma_start(
                        st[:], xs[n][:, lo * W:hi * W])
                return st

            def quant_scale(n, c, st):
                # gpsimd: a = 63.75*x + 2^23 in-place (f32 RNE to integer)
                nc.gpsimd.tensor_scalar(st[:], st[:], 63.75, float(MAGIC),
                                        op0=mult, op1=add)

            def quant_conv(n, c, st):
                # ACT: d = a - 2^23 cast to fp16, into padded d rows.
                # (gpsimd is catastrophic here: strided fp16 writes hit a
                # ~12 cyc/elem Q7 path AND starve the DVE via the shared
                # SBUF port.)
                s = n % 2
                lo, hi = SLAB_LO[c], SLAB_HI[c]
                nr = hi - lo
                d3 = dts[s].rearrange("p (h w) -> p h w", w=WP)
                dst = d3[:, 1 + lo: 1 + hi, 1:1 + W]
                src = st.rearrange("p (h w) -> p h w", w=W)[:, :nr, :]
                nc.scalar.activation(dst, src, Copy, bias=float(-MAGIC))

            def fwd(n, c, nt=CH, js=(0, 1, 2, 3)):
                # V_j[i] for htiles i in [i0, i0+nt) from padded rows
                # 2*i0..2*(i0+nt)+1.  d4[p, i, r, w]: padded row 2i+r.
                s = n % 2
                d4 = dts[s].rearrange("p (i r w) -> p i r w", r=2, w=WP)
                v = [vts[s][j].rearrange("p (i w) -> p i w", w=WP)
                     for j in range(NPOS)]
                i0 = CH * c
                e0 = d4[:, i0:i0 + nt, 0, :]        # rows 2i
                e1 = d4[:, i0 + 1:i0 + nt + 1, 0, :]  # rows 2i+2
                o0 = d4[:, i0:i0 + nt, 1, :]        # rows 2i+1
                o1 = d4[:, i0 + 1:i0 + nt + 1, 1, :]  # rows 2i+3
                srcs = {0: (e0, e1, sub), 1: (o0, e1, add),
                        2: (e1, o0, sub), 3: (o0, o1, sub)}
                for j in js:
                    a, b, op = srcs[j]
                    nc.vector.tensor_tensor(v[j][:, i0:i0 + nt, :], a, b, op)

            ot_tiles = {}
            ps_seq = 0

            def compute(n, c, ct):
                nonlocal ps_seq
                s = n % 2
                half = c // 2
                key = (ct, half)
                if c % 2 == 0:
                    ot_tiles[key] = opool.tile([128, 2 * CH * 2 * W], f32,
                                               tag=f"ot{ct}",
                                               name=f"ot{n}_{ct}_{half}")
                ot = ot_tiles[key]
                i0 = CH * c
                ps = []
                bank0 = (ps_seq % 2) * 4
                ps_seq += 1
                for j in range(NPOS):
                    p = pspool.tile([128, NMM], f32, tag=f"ps{bank0 + j}",
                                    name=f"ps{n}_{c}_{ct}_{j}")
                    ps.append(p)
                    vv = vts[s][j].rearrange("p (i w) -> p i w", w=WP)
                    for kw in range(KW):
                        nc.tensor.matmul(
                            p[:], wmat(j, kw, ct),
                            vv[:, i0:i0 + CH, kw:kw + W],
                            start=(kw == 0), stop=(kw == KW - 1),
                        )
                m0p = mpool.tile([128, NMM], f16, tag="m0p",
                                 name=f"m0p{n}_{c}_{ct}")
                nc.scalar.activation(m0p[:], ps[0][:], Copy)
                m1p = mpool.tile([128, NMM], f16, tag="m1p",
                                 name=f"m1p{n}_{c}_{ct}")
                nc.scalar.activation(m1p[:], ps[1][:], Ident,
                                     bias=shb[:, ct:ct + 1])
                m2p = mpool.tile([128, NMM], f16, tag="m2p",
                                 name=f"m2p{n}_{c}_{ct}")
                nc.scalar.activation(m2p[:], ps[2][:], Copy)
                s01 = mpool.tile([128, NMM], f16, tag="s01",
                                 name=f"s01{n}_{c}_{ct}")
                nc.vector.tensor_tensor(s01[:], m0p[:], m1p[:], add)
                u = mpool.tile([128, NMM], f16, tag="u",
                               name=f"u{n}_{c}_{ct}")
                nc.vector.tensor_tensor(u[:], m1p[:], m2p[:], sub)
                # ot view [p, chunkhalf, htile, evenodd, w]
                o4 = ot.rearrange("p (k i r w) -> p k i r w", k=2, r=2, w=W)
                nc.gpsimd.tensor_tensor(o4[:, c % 2, :, 0, :], s01[:],
                                        m2p[:], add)
                nc.vector.tensor_tensor(o4[:, c % 2, :, 1, :], u[:],
                                        ps[3][:], sub)

            store_seq = 0
            CHW = CH * 2 * W  # 784, one chunk's output elements

            def store(n, c, ct, whole_half):
                nonlocal store_seq
                half = c // 2
                ot = ot_tiles[(ct, half)]
                if whole_half:
                    src = ot[:]
                    dst = ys[n, ct * 128:(ct + 1) * 128,
                             half * 2 * CHW:(half + 1) * 2 * CHW]
                    dma_engs[store_seq % 3].dma_start(dst, src)
                else:
                    # final image: split each chunk store across two queues
                    src = ot[:, (c % 2) * CHW:(c % 2 + 1) * CHW]
                    h = CHW // 2
                    o = c * CHW
                    cs = ct * 128
                    dma_engs[store_seq % 3].dma_start(
                        ys[n, cs:cs + 128, o:o + h], src[:, :h])
                    dma_engs[(store_seq + 1) % 3].dma_start(
                        ys[n, cs:cs + 128, o + h:o + 2 * h], src[:, h:])
                store_seq += 1

            # ---- emission ----------------------------------------------
            # image 0 prep.  HWDGE rings are FIFO per engine, so the trigger
            # order IS the priority order: slab 0 halves first on both rings,
            # then the weights, then the remaining slabs.
            sts = {}
            sts[(0, 0)] = dma_slab(0, 0, split=True)
            nc.sync.dma_start(w1[:], wb[:, :6 * 128])
            nc.scalar.dma_start(w2[:], wb[:, 6 * 128:])
            sts[(0, 1)] = dma_slab(0, 1, split=False)   # scalar
            sts[(0, 2)] = dma_slab(0, 2, split=False)   # sync
            sts[(0, 3)] = dma_slab(0, 3, split=False)   # scalar
            for c in range(NCH):
                quant_scale(0, c, sts[(0, c)])
                quant_conv(0, c, sts[(0, c)])
                fwd(0, c)

            for n in range(IMGS):
                last = n == IMGS - 1
                for c in range(NCH):
                    if not last and c < 2:
                        # next image's input slabs, two per compute chunk
                        sts[(n + 1, 2 * c)] = dma_slab(n + 1, 2 * c, False)
                        sts[(n + 1, 2 * c + 1)] = dma_slab(n + 1, 2 * c + 1,
                                                           False)
                    compute(n, c, 0)
                    if not last:
                        quant_scale(n + 1, c, sts[(n + 1, c)])
                        if c == 2:
                            fwd(n + 1, 0, nt=2 * CH, js=(0, 1))
                    compute(n, c, 1)
                    if not last:
                        quant_conv(n + 1, c, sts[(n + 1, c)])
                        if c == 2:
                            fwd(n + 1, 0, nt=2 * CH, js=(2, 3))
                        elif c == 3:
                            fwd(n + 1, 2, nt=2 * CH)
                    if last:
                        store(n, c, 0, whole_half=False)
                        store(n, c, 1, whole_half=False)
                    elif c % 2 == 1:
                        store(n, c, 0, whole_half=True)
                        store(n, c, 1, whole_half=True)

    nc.compile()
    return nc


def _get_nc():
    global _cached_nc
    if _cached_nc is None:
        _cached_nc = _build()
    return _cached_nc


def _host_prep(weight, bias, gamma, beta, running_mean, running_var):
    # DoReFa weight quantization mirroring the reference.
    wt = np.tanh(weight)
    wt = wt / np.abs(wt).max()
    k = np.round(wt * np.float32(127.5) + np.float32(127.5)).astype(np.float32)
    b_int = np.float32(2.0) * k - np.float32(255.0)  # [COUT, CIN, 3, 3]

    inv = gamma * (np.float32(1.0) / np.sqrt(running_var + np.float32(1e-5)))
    shift = beta - running_mean * inv + bias
    scale = inv * np.float32(4.0 / 65025.0)

    g0 = b_int[:, :, 0, :]
    g1 = b_int[:, :, 1, :]
    g2 = b_int[:, :, 2, :]
    U = np.stack([g0, (g0 + g1 + g2) * np.float32(0.5),
                  (g0 - g1 + g2) * np.float32(0.5), g2], axis=0)
    U = U * scale[None, :, None, None]            # [4, COUT, CIN, KW]
    wb_host = np.zeros((CIN, NPOS * KW * COUT), dtype=np.float16)
    for j in range(NPOS):
        for kw in range(KW):
            for ct in range(COUT_TILES):
                idx = (j * KW + kw) * 2 + ct
                wb_host[:, idx * 128:(idx + 1) * 128] = (
                    U[j, ct * 128:(ct + 1) * 128, :, kw].T
                )
    sh_host = np.ascontiguousarray(shift.reshape(COUT_TILES, 128).T)
    return wb_host, sh_host


def kernel(x, weight, bias, gamma, beta, running_mean, running_var):
    global LAST_RESULT
    x = np.asarray(x, dtype=np.float32)
    wb_host, sh_host = _host_prep(
        np.asarray(weight, dtype=np.float32),
        np.asarray(bias, dtype=np.float32),
        np.asarray(gamma, dtype=np.float32),
        np.asarray(beta, dtype=np.float32),
        np.asarray(running_mean, dtype=np.float32),
        np.asarray(running_var, dtype=np.float32))

    nc = _get_nc()
    in_maps = []
    for core in range(N_CORES):
        xs_c = np.ascontiguousarray(
            x[core * IMGS:(core + 1) * IMGS].reshape(IMGS, CIN, HW)
        )
        in_maps.append({"xs": xs_c, "wb": wb_host, "shd": sh_host})

    res = run_bass_kernel_spmd(nc, in_maps, list(range(N_CORES)), trace=TRACE,
                               tmpdir=TRACE_DIR)
    LAST_RESULT = res

    out = np.empty((N_BATCH, COUT, H, W), dtype=np.float32)
    for core in range(N_CORES):
        out[core * IMGS:(core + 1) * IMGS] = (
            res.results[core]["ys"].reshape(IMGS, COUT, H, W)
        )
    return out


# revision 7
# speedup vs baseline: 1.8379x; 1.0136x over previous
"""Fused quantized Conv2D + BatchNorm via 1-D Winograd F(2,3) on Trainium2.

Reference computation (shapes hardcoded):
  x:      [32, 128, 56, 56] f32    activations in [0, 1)
  weight: [256, 128, 3, 3]  f32
  bias/gamma/beta/running_mean/running_var: [256] f32

  xq = round(clip(x,0,4) * 255/4) * (4/255)          (8-bit act quant)
  wq = DoReFa 8-bit weight quant -> values (2k-255)/255
  y  = conv2d(xq, wq, stride 1, pad 1)               NCHW/OIHW
  out = y * inv + shift

Kernel strategy:
  - Data-parallel over batch: core i handles images [4i, 4i+4).
  - Winograd F(2,3) applied over the H axis: each pair of output rows
    (one "htile") costs 4 position-matmuls x 3 horizontal taps = 12
    matmuls per (chunk, couttile) instead of the 18 a direct conv
    needs: 1.5x less PE work.  All values stay exact in fp16:
    d = round(63.75*x) in {0..255}; V = B^T d ints <= 510; U = G b
    half-ints <= 382.5 scaled by the folded BN scale.
  - Inverse transform y_e = m0+m1+m2+sh, y_o = m1-m2-m3+sh is spread
    over three engines (PSUM has one DVE read port, so each op reads
    at most one PSUM operand):
      ACT:   m1p = m1 + sh -> fp16,  m2p = m2 -> fp16
      DVE:   s01 = m0(PSUM) + m1p,   u = m1p - m2p (2x fp16),
             y_o = u - m3(PSUM) -> f32
      GpSimd: y_e = s01 + m2p -> f32
  - Quantization runs on ACT (Copy with scale=63.75, bias=2^23 forcing
    RNE to integers, then Copy with bias=-2^23 casting to fp16 d).
  - Forward transform: 4 fp16 tensor_tensor ops per chunk on DVE at
    2x packing (V_j tiles carry the 58-wide column halo so the 3 kw
    taps are strided views).
"""

import sys
import types

import numpy as np
import ml_dtypes

import concourse.bacc as bacc
import concourse.tile as tile
from concourse import mybir
from concourse.bass_utils import run_bass_kernel_spmd
from concourse.vector_clock import ScopedClock


def _ensure_axon_hooks_shim():
    import antenv
    if hasattr(antenv, "axon_hooks"):
        return
    mod = types.ModuleType("antenv.axon_hooks")
    _hook = [None]
    mod.set_axon_ntff_profile_hook = lambda h: _hook.__setitem__(0, h)
    mod.get_axon_ntff_profile_hook = lambda: _hook[0]
    sys.modules["antenv.axon_hooks"] = mod
    antenv.axon_hooks = mod


_ensure_axon_hooks_shim()


class _FastBacc(bacc.Bacc):
    """Bacc whose constructor-time all-engine barrier is skipped."""

    _skip_one_barrier = True

    def all_engine_barrier(self, *, sem_only: bool = False):
        if self._skip_one_barrier:
            self._skip_one_barrier = False
            return
        super().all_engine_barrier(sem_only=sem_only)


class _FastExitTileContext(tile.TileContext):
    """TileContext with a cheaper exit ceremony (see baseline notes)."""

    def _drain_and_barrier(self, tick_clock, wait_clock):
        drain_inst = self.nc.gpsimd.drain()
        wait_clock.add_sem_waits(
            drain_inst.ins, ScopedClock({None: tick_clock.global_clock})
        )
        popped = self.nc._tile_sem_poison_stack.pop()
        assert popped is self._sem_poison
        self.nc.clear_and_free_semaphores(list(self.sems.allocated().values()))


N_CORES = 8
N_BATCH = 32
IMGS = N_BATCH // N_CORES
CIN = 128
COUT = 256
H = W = 56
HW = H * W
WP = 58          # padded column width (col halo for the 3 kw taps)
HP = 58          # padded row count (rows -1 .. 56)
HT = 28          # htiles per image (2 output rows each)
CH = 7           # htiles per matmul chunk
NCH = HT // CH   # 4 chunks
NMM = CH * W     # 392 free elements per matmul
COUT_TILES = 2
NPOS = 4         # Winograd positions j
KW = 3
N_WARM_MM = 22

# disjoint input slabs: slab c covers x rows [SLAB_LO[c], SLAB_HI[c])
SLAB_LO = [0, 15, 29, 43]
SLAB_HI = [15, 29, 43, 56]

MAGIC = np.float32(2.0**23)

TRACE = False
TRACE_DIR = None
LAST_RESULT = None
SIM_SAFE_EXIT = False  # use the stock (slow, race-clean) exit for CoreSim

_cached_nc = None


def _build():
    f32 = mybir.dt.float32
    f16 = mybir.dt.float16
    mult = mybir.AluOpType.mult
    add = mybir.AluOpType.add
    sub = mybir.AluOpType.subtract
    Copy = mybir.ActivationFunctionType.Copy
    Ident = mybir.ActivationFunctionType.Identity

    ctx_cls = tile.TileContext if SIM_SAFE_EXIT else _FastExitTileContext
    bacc_cls = bacc.Bacc if SIM_SAFE_EXIT else _FastBacc
    nc = bacc_cls("TRN2", target_bir_lowering=False, debug=False,
                  num_devices=N_CORES)
    xs = nc.dram_tensor("xs", [IMGS, CIN, HW], f32, kind="ExternalInput").ap()
    # Winograd weights, column idx = ((j*3+kw)*2 + ct)*128 + cout_local
    wb = nc.dram_tensor("wb", [CIN, NPOS * KW * COUT], f16,
                        kind="ExternalInput").ap()
    shd = nc.dram_tensor("shd", [128, COUT_TILES], f32, kind="ExternalInput").ap()
    ys = nc.dram_tensor("ys", [IMGS, COUT, HW], f32, kind="ExternalOutput").ap()

    with ctx_cls(nc) as tc:
        with (
            tc.tile_pool(name="wpool", bufs=1) as wpool,
            tc.tile_pool(name="ppool", bufs=1) as ppool,
            tc.tile_pool(name="stpool", bufs=4) as stpool,
            tc.tile_pool(name="dpool", bufs=1) as dpool,
            tc.tile_pool(name="vpool", bufs=1) as vpool,
            tc.tile_pool(name="mpool", bufs=3) as mpool,
            tc.tile_pool(name="opool", bufs=5) as opool,
            tc.tile_pool(name="pspool", bufs=1, space="PSUM") as pspool,
        ):
            # ACT warm-up: the activation-table load overlaps the first DMA.
            warm = ppool.tile([128, 1], f32, tag="warm")
            nc.vector.memset(warm[:], 0.0)
            nc.scalar.activation(warm[:], warm[:], Copy, bias=0.0)

            # HAM warm-up dummies.
            dummy = wpool.tile([128, 128 + NMM], f16, tag="dummy")
            nc.vector.memset(dummy[:], 0.0)
            dps = pspool.tile([128, NMM], f32, tag="ps0", name="ps_dummy")
            for i in range(N_WARM_MM):
                nc.tensor.matmul(dps[:], dummy[:, :128], dummy[:, 128:],
                                 start=True, stop=True)

            # shift vector [128, 2]
            shb = ppool.tile([128, COUT_TILES], f32, tag="shb")
            nc.gpsimd.dma_start(shb[:], shd[:])

            # d tiles (2 image slots), fp16; only the pad borders (row 0,
            # row 57, col 0, col 57) need zeroing -- interiors are fully
            # rewritten every image.
            dts = []
            for s in range(2):
                t = dpool.tile([CIN, HP * WP], f16, tag=f"d{s}", name=f"d{s}")
                eng = nc.gpsimd if s == 0 else nc.vector
                t3 = t.rearrange("p (h w) -> p h w", w=WP)
                eng.memset(t3[:, 0, :], 0.0)       # top pad row
                eng.memset(t3[:, HP - 1, :], 0.0)  # bottom pad row
                eng.memset(t3[:, :, 0:1], 0.0)     # left pad col
                eng.memset(t3[:, :, WP - 1:WP], 0.0)  # right pad col
                dts.append(t)
            # V tiles (4 positions x 2 slots)
            vts = [[vpool.tile([CIN, HT * WP], f16, tag=f"v{j}_{s}",
                               name=f"v{j}_{s}")
                    for j in range(NPOS)] for s in range(2)]

            # weights: j=0 block first (needed by the first matmul group)
            w1 = wpool.tile([CIN, 6 * 128], f16, tag="w1")
            w2 = wpool.tile([CIN, 18 * 128], f16, tag="w2")

            def wmat(j, kw, ct):
                idx = (j * KW + kw) * 2 + ct
                if j == 0:
                    return w1[:, idx * 128:(idx + 1) * 128]
                return w2[:, (idx - 6) * 128:(idx - 6 + 1) * 128]

            dma_engs = [nc.sync, nc.scalar, nc.gpsimd]

            # ---- helpers ------------------------------------------------
            def dma_slab(n, c, split):
                lo, hi = SLAB_LO[c], SLAB_HI[c]
                nr = hi - lo
                st = stpool.tile([CIN, nr * W], f32, tag="st",
                                 name=f"st{n}_{c}")
                if split:
                    h = (nr // 2) * W
                    nc.sync.dma_start(st[:, :h], xs[n][:, lo * W:lo * W + h])
                    nc.scalar.dma_start(st[:, h:],
                                        xs[n][:, lo * W + h:hi * W])
                else:
                    (nc.sync if c % 2 == 0 else nc.scalar).dma_start(
                        st[:], xs[n][:, lo * W:hi * W])
                return st

            def quant_scale(n, c, st):
                # gpsimd: a = 63.75*x + 2^23 in-place (f32 RNE to integer)
                nc.gpsimd.tensor_scalar(st[:], st[:], 63.75, float(MAGIC),
                                        op0=mult, op1=add)

            def quant_conv(n, c, st):
                # ACT: d = a - 2^23 cast to fp16, into padded d rows.
                # (gpsimd is catastrophic here: strided fp16 writes hit a
                # ~12 cyc/elem Q7 path AND starve the DVE via the shared
                # SBUF port.)
                s = n % 2
                lo, hi = SLAB_LO[c], SLAB_HI[c]
                nr = hi - lo
                d3 = dts[s].rearrange("p (h w) -> p h w", w=WP)
                dst = d3[:, 1 + lo: 1 + hi, 1:1 + W]
                src = st.rearrange("p (h w) -> p h w", w=W)[:, :nr, :]
                nc.scalar.activation(dst, src, Copy, bias=float(-MAGIC))

            def fwd(n, c, nt=CH, js=(0, 1, 2, 3)):
                # V_j[i] for htiles i in [i0, i0+nt) from padded rows
                # 2*i0..2*(i0+nt)+1.  d4[p, i, r, w]: padded row 2i+r.
                s = n % 2
                d4 = dts[s].rearrange("p (i r w) -> p i r w", r=2, w=WP)
                v = [vts[s][j].rearrange("p (i w) -> p i w", w=WP)
                     for j in range(NPOS)]
                i0 = CH * c
                e0 = d4[:, i0:i0 + nt, 0, :]        # rows 2i
                e1 = d4[:, i0 + 1:i0 + nt + 1, 0, :]  # rows 2i+2
                o0 = d4[:, i0:i0 + nt, 1, :]        # rows 2i+1
                o1 = d4[:, i0 + 1:i0 + nt + 1, 1, :]  # rows 2i+3
                srcs = {0: (e0, e1, sub), 1: (o0, e1, add),
                        2: (e1, o0, sub), 3: (o0, o1, sub)}
                for j in js:
                    a, b, op = srcs[j]
                    nc.vector.tensor_tensor(v[j][:, i0:i0 + nt, :], a, b, op)

            ot_tiles = {}
            ps_seq = 0

            def compute(n, c, ct):
                nonlocal ps_seq
                s = n % 2
                half = c // 2
                key = (ct, half)
                if c % 2 == 0:
                    ot_tiles[key] = opool.tile([128, 2 * CH * 2 * W], f32,
                                               tag=f"ot{ct}",
                                               name=f"ot{n}_{ct}_{half}")
                ot = ot_tiles[key]
                i0 = CH * c
                ps = []
                bank0 = (ps_seq % 2) * 4
                ps_seq += 1
                for j in range(NPOS):
                    p = pspool.tile([128, NMM], f32, tag=f"ps{bank0 + j}",
                                    name=f"ps{n}_{c}_{ct}_{j}")
                    ps.append(p)
                    vv = vts[s][j].rearrange("p (i w) -> p i w", w=WP)
                    for kw in range(KW):
                        nc.tensor.matmul(
                            p[:], wmat(j, kw, ct),
                            vv[:, i0:i0 + CH, kw:kw + W],
                            start=(kw == 0), stop=(kw == KW - 1),
                        )
                m1p = mpool.tile([128, NMM], f16, tag="m1p",
                                 name=f"m1p{n}_{c}_{ct}")
                nc.scalar.activation(m1p[:], ps[1][:], Ident,
                                     bias=shb[:, ct:ct + 1])
                m2p = mpool.tile([128, NMM], f16, tag="m2p",
                                 name=f"m2p{n}_{c}_{ct}")
                nc.scalar.activation(m2p[:], ps[2][:], Copy)
                s01 = mpool.tile([128, NMM], f16, tag="s01",
                                 name=f"s01{n}_{c}_{ct}")
                nc.vector.tensor_tensor(s01[:], ps[0][:], m1p[:], add)
                u = mpool.tile([128, NMM], f16, tag="u",
                               name=f"u{n}_{c}_{ct}")
                nc.vector.tensor_tensor(u[:], m1p[:], m2p[:], sub)
                # ot view [p, chunkhalf, htile, evenodd, w]
                o4 = ot.rearrange("p (k i r w) -> p k i r w", k=2, r=2, w=W)
                nc.gpsimd.tensor_tensor(o4[:, c % 2, :, 0, :], s01[:],
                                        m2p[:], add)
                nc.vector.tensor_tensor(o4[:, c % 2, :, 1, :], u[:],
                                        ps[3][:], sub)

            store_seq = 0
            CHW = CH * 2 * W  # 784, one chunk's output elements

            def store(n, c, ct, whole_half):
                nonlocal store_seq
                half = c // 2
                ot = ot_tiles[(ct, half)]
                if whole_half:
                    src = ot[:]
                    dst = ys[n, ct * 128:(ct + 1) * 128,
                             half * 2 * CHW:(half + 1) * 2 * CHW]
                    dma_engs[store_seq % 3].dma_start(dst, src)
                else:
                    # final image: split each chunk store across two queues
                    src = ot[:, (c % 2) * CHW:(c % 2 + 1) * CHW]
                    h = CHW // 2
                    o = c * CHW
                    cs = ct * 128
                    dma_engs[store_seq % 3].dma_start(
                        ys[n, cs:cs + 128, o:o + h], src[:, :h])
                    dma_engs[(store_seq + 1) % 3].dma_start(
                        ys[n, cs:cs + 128, o + h:o + 2 * h], src[:, h:])
                store_seq += 1

            # ---- emission ----------------------------------------------
            # image 0 prep.  HWDGE rings are FIFO per engine, so the trigger
            # order IS the priority order: slab 0 halves first on both rings,
            # then the weights, then the remaining slabs.
            sts = {}
            sts[(0, 0)] = dma_slab(0, 0, split=True)
            nc.sync.dma_start(w1[:], wb[:, :6 * 128])
            nc.scalar.dma_start(w2[:], wb[:, 6 * 128:])
            sts[(0, 1)] = dma_slab(0, 1, split=False)   # scalar
            sts[(0, 2)] = dma_slab(0, 2, split=False)   # sync
            sts[(0, 3)] = dma_slab(0, 3, split=False)   # scalar
            for c in range(NCH):
                quant_scale(0, c, sts[(0, c)])
                quant_conv(0, c, sts[(0, c)])
                fwd(0, c)

            for n in range(IMGS):
                last = n == IMGS - 1
                for c in range(NCH):
                    if not last and c < 2:
                        # next image's input slabs, two per compute chunk
                        sts[(n + 1, 2 * c)] = dma_slab(n + 1, 2 * c, False)
                        sts[(n + 1, 2 * c + 1)] = dma_slab(n + 1, 2 * c + 1,
                                                           False)
                    compute(n, c, 0)
                    if not last:
                        quant_scale(n + 1, c, sts[(n + 1, c)])
                        if c == 2:
                            fwd(n + 1, 0, nt=2 * CH, js=(0, 1))
                    compute(n, c, 1)
                    if not last:
                        quant_conv(n + 1, c, sts[(n + 1, c)])
                        if c == 2:
                            fwd(n + 1, 0, nt=2 * CH, js=(2, 3))
                        elif c == 3:
                            fwd(n + 1, 2, nt=2 * CH)
                    if last:
                        store(n, c, 0, whole_half=False)
                        store(n, c, 1, whole_half=False)
                    elif c % 2 == 1:
                        store(n, c, 0, whole_half=True)
                        store(n, c, 1, whole_half=True)

    nc.compile()
    return nc


def _get_nc():
    global _cached_nc
    if _cached_nc is None:
        _cached_nc = _build()
    return _cached_nc


def _host_prep(weight, bias, gamma, beta, running_mean, running_var):
    # DoReFa weight quantization mirroring the reference.
    wt = np.tanh(weight)
    wt = wt / np.abs(wt).max()
    k = np.round(wt * np.float32(127.5) + np.float32(127.5)).astype(np.float32)
    b_int = np.float32(2.0) * k - np.float32(255.0)  # [COUT, CIN, 3, 3]

    inv = gamma * (np.float32(1.0) / np.sqrt(running_var + np.float32(1e-5)))
    shift = beta - running_mean * inv + bias
    scale = inv * np.float32(4.0 / 65025.0)

    g0 = b_int[:, :, 0, :]
    g1 = b_int[:, :, 1, :]
    g2 = b_int[:, :, 2, :]
    U = np.stack([g0, (g0 + g1 + g2) * np.float32(0.5),
                  (g0 - g1 + g2) * np.float32(0.5), g2], axis=0)
    U = U * scale[None, :, None, None]            # [4, COUT, CIN, KW]
    wb_host = np.zeros((CIN, NPOS * KW * COUT), dtype=np.float16)
    for j in range(NPOS):
        for kw in range(KW):
            for ct in range(COUT_TILES):
                idx = (j * KW + kw) * 2 + ct
                wb_host[:, idx * 128:(idx + 1) * 128] = (
                    U[j, ct * 128:(ct + 1) * 128, :, kw].T
                )
    sh_host = np.ascontiguousarray(shift.reshape(COUT_TILES, 128).T)
    return wb_host, sh_host


def kernel(x, weight, bias, gamma, beta, running_mean, running_var):
    global LAST_RESULT
    x = np.asarray(x, dtype=np.float32)
    wb_host, sh_host = _host_prep(
        np.asarray(weight, dtype=np.float32),
        np.asarray(bias, dtype=np.float32),
        np.asarray(gamma, dtype=np.float32),
        np.asarray(beta, dtype=np.float32),
        np.asarray(running_mean, dtype=np.float32),
        np.asarray(running_var, dtype=np.float32))

    nc = _get_nc()
    in_maps = []
    for core in range(N_CORES):
        xs_c = np.ascontiguousarray(
            x[core * IMGS:(core + 1) * IMGS].reshape(IMGS, CIN, HW)
        )
        in_maps.append({"xs": xs_c, "wb": wb_host, "shd": sh_host})

    res = run_bass_kernel_spmd(nc, in_maps, list(range(N_CORES)), trace=TRACE,
                               tmpdir=TRACE_DIR)
    LAST_RESULT = res

    out = np.empty((N_BATCH, COUT, H, W), dtype=np.float32)
    for core in range(N_CORES):
        out[core * IMGS:(core + 1) * IMGS] = (
            res.results[core]["ys"].reshape(IMGS, COUT, H, W)
        )
    return out
